# revision 1
# baseline (speedup 1.0000x reference)
"""BPaCo+ loss on 8 TRN2 NeuronCores.

Strategy: shard the [B+K] contrast-column dimension across 8 cores. Each core
computes, for all B=1024 anchor rows, the partial softmax-denominator sums of
its 4224-column shard via a PE-only pipeline:

    P[j,i] = f_j . f_i / T   (matmul, bf16)
    E      = exp(P)          (scalar engine, PSUM -> SBUF bf16)
    acc[0,i]   += sum_j E[j,i] * r0_j          (matmul, lhsT col 0 = r0)
    acc[1+c,i] += sum_j E[j,i] * d_j * 1[lab_j==c]   (matmul, lhsT = scaled onehot)

which encodes the per-instance class weights w = cls_count[lab_j] - mask*ALPHA
as 1/w = r0_j + m_ij*d_j. The mask-weighted logit sums A_i reduce to
f_i . g_{lab_i} / T with g_c = class-summed features (host precompute).
Branch 1 (anchors vs batch+centers) and the sup_logits block run through the
same uniform slot pipeline with per-core data (zero weights on cores that
don't own them), keeping the SPMD instruction stream identical on all cores.
Per-row partials [2,1024] + A vectors are DMA'd out; the final O(B) log/mean
runs on host (the "mean all-reduce" gather step).
"""
import numpy as np
import ml_dtypes

from concourse import bass, bacc, mybir, tile
from concourse.bass_utils import run_bass_kernel_spmd

B, K, C, D = 1024, 32768, 100, 128
N = B + K
T, ALPHA = 0.07, 0.05
M = 8                      # cores
SHARD = N // M             # 4224 columns per core
JC_B2 = SHARD // 128       # 33 main slots
N_SLOT = JC_B2 + 3         # + slot33 (b1 feats), slot34 (b1 extra), slot35 (sup)
BF16 = mybir.dt.bfloat16
F32 = mybir.dt.float32
NP_BF16 = ml_dtypes.bfloat16

_CACHE = {}


def _build_nc():
    nc = bacc.Bacc(None, target_bir_lowering=False)
    fbT = nc.declare_dram_parameter("fbT", [D, B], BF16, isOutput=False)
    fTs = nc.declare_dram_parameter("fTs", [D, N_SLOT * 128], BF16, isOutput=False)
    W = nc.declare_dram_parameter("W", [128, N_SLOT * 101], BF16, isOutput=False)
    antidiag = nc.declare_dram_parameter("antidiag", [128, 128], BF16, isOutput=False)
    maskfull = nc.declare_dram_parameter("maskfull", [128, 3 * B], BF16, isOutput=False)
    gT = nc.declare_dram_parameter("gT", [D, 2 * C], BF16, isOutput=False)
    iota_b = nc.declare_dram_parameter("iota_b", [128, C], F32, isOutput=False)
    rowlab = nc.declare_dram_parameter("rowlab", [128, 8], F32, isOutput=False)
    labB = nc.declare_dram_parameter("labB", [128, B], F32, isOutput=False)
    iota_col = nc.declare_dram_parameter("iota_col", [128, 1], F32, isOutput=False)
    partS = nc.declare_dram_parameter("partS", [1, 2 * B], F32, isOutput=True)
    partA = nc.declare_dram_parameter("partA", [128, 16], F32, isOutput=True)

    with tile.TileContext(nc) as tc:
        with (
            tc.tile_pool(name="const", bufs=1) as cpool,
            tc.tile_pool(name="epool", bufs=4) as epool,
            tc.tile_pool(name="small", bufs=4) as spool,
            tc.tile_pool(name="mainps", bufs=2, space=bass.MemorySpace.PSUM) as mps,
            tc.tile_pool(name="accps", bufs=1, space=bass.MemorySpace.PSUM) as aps,
        ):
            # resident inputs
            fbT_sb = cpool.tile([D, B], BF16, tag="fbT")
            nc.sync.dma_start(fbT_sb[:], fbT[:])
            fTs_sb = cpool.tile([D, N_SLOT * 128], BF16, tag="fTs")
            nc.sync.dma_start(fTs_sb[:], fTs[:])
            W_sb = cpool.tile([128, N_SLOT * 101], BF16, tag="W")
            nc.sync.dma_start(W_sb[:], W[:])
            ad_sb = cpool.tile([128, 128], BF16, tag="ad")
            nc.sync.dma_start(ad_sb[:], antidiag[:])
            mf_sb = cpool.tile([128, 3 * B], BF16, tag="mf")
            nc.sync.dma_start(mf_sb[:], maskfull[:])
            gT_sb = cpool.tile([D, 2 * C], BF16, tag="gT")
            nc.sync.dma_start(gT_sb[:], gT[:])
            iob_sb = cpool.tile([128, C], F32, tag="iob")
            nc.sync.dma_start(iob_sb[:], iota_b[:])
            rl_sb = cpool.tile([128, 8], F32, tag="rl")
            nc.sync.dma_start(rl_sb[:], rowlab[:])
            lB_sb = cpool.tile([128, B], F32, tag="lB")
            nc.sync.dma_start(lB_sb[:], labB[:])
            ioc_sb = cpool.tile([128, 1], F32, tag="ioc")
            nc.sync.dma_start(ioc_sb[:], iota_col[:])
            ones_sb = cpool.tile([128, 1], F32, tag="ones")
            nc.vector.memset(ones_sb[:], 1.0)

            acc2 = aps.tile([101, B], F32, tag="acc2")   # 2 banks
            acc1 = aps.tile([101, B], F32, tag="acc1")   # 2 banks

            # ---- main slot pipeline ----
            for jc in range(N_SLOT):
                for h in range(2):
                    P = mps.tile([128, 512], F32, tag="P")
                    nc.tensor.matmul(
                        P[:],
                        fTs_sb[:, jc * 128:(jc + 1) * 128],
                        fbT_sb[:, h * 512:(h + 1) * 512],
                        start=True, stop=True,
                    )
                    E = epool.tile([128, 512], BF16, tag="E")
                    nc.scalar.activation(E[:], P[:], mybir.ActivationFunctionType.Exp)
                    if jc < 8:
                        # diagonal block of the b2 shard (only core 0's data
                        # has it; other cores receive an all-ones tile)
                        hd = (jc * 128) // 512
                        if h == hd:
                            o = jc * 128 - hd * 512
                            nc.vector.tensor_tensor(
                                E[:, o:o + 128], E[:, o:o + 128], ad_sb[:],
                                op=mybir.AluOpType.mult,
                            )
                    if jc >= JC_B2:
                        s = jc - JC_B2
                        nc.vector.tensor_tensor(
                            E[:], E[:], mf_sb[:, s * B + h * 512: s * B + (h + 1) * 512],
                            op=mybir.AluOpType.mult,
                        )
                    if jc in (JC_B2, JC_B2 + 1):
                        accT, first, last = acc1, jc == JC_B2, jc == JC_B2 + 1
                    else:
                        accT = acc2
                        first = jc == 0
                        last = jc == N_SLOT - 1
                    nc.tensor.matmul(
                        accT[:, h * 512:(h + 1) * 512],
                        W_sb[:, jc * 101:(jc + 1) * 101],
                        E[:],
                        start=first, stop=last,
                    )

            # ---- A vectors: A[i] = f_i . g_{lab_i} / T  (1/T folded in fbT) ----
            Aout = spool.tile([128, 16], F32, tag="Aout")
            for ic in range(8):
                PA = mps.tile([128, 512], F32, tag="P")
                nc.tensor.matmul(
                    PA[:, :2 * C],
                    fbT_sb[:, ic * 128:(ic + 1) * 128],
                    gT_sb[:],
                    start=True, stop=True,
                )
                oh = spool.tile([128, C], F32, tag="oh")
                nc.vector.tensor_scalar(
                    oh[:], iob_sb[:], rl_sb[:, ic:ic + 1], None,
                    op0=mybir.AluOpType.is_equal,
                )
                t2 = spool.tile([128, C], F32, tag="t2")
                nc.vector.tensor_tensor(t2[:], oh[:], PA[:, :C], op=mybir.AluOpType.mult)
                nc.vector.tensor_reduce(
                    Aout[:, ic:ic + 1], t2[:], axis=mybir.AxisListType.X,
                    op=mybir.AluOpType.add,
                )
                t3 = spool.tile([128, C], F32, tag="t3")
                nc.vector.tensor_tensor(t3[:], oh[:], PA[:, C:2 * C], op=mybir.AluOpType.mult)
                nc.vector.tensor_reduce(
                    Aout[:, 8 + ic:9 + ic], t3[:], axis=mybir.AxisListType.X,
                    op=mybir.AluOpType.add,
                )
            nc.sync.dma_start(partA[:], Aout[:])

            # ---- S_i = acc[0, i] + acc[1+lab_i, i]  via shifted-onehot select ----
            # ohb[p, i] = 1 if p == 0 else (labB[i] == p-1); iota_col holds p-1
            ohb = spool.tile([128, B], F32, tag="ohb")
            nc.vector.tensor_scalar(
                ohb[:], lB_sb[:], ioc_sb[:], None, op0=mybir.AluOpType.is_equal,
            )
            nc.vector.memset(ohb[0:1, :], 1.0)
            sel2 = spool.tile([128, B], F32, tag="sel2")
            nc.vector.tensor_tensor(sel2[:101, :], acc2[0:101, :], ohb[:101, :], op=mybir.AluOpType.mult)
            sel1 = spool.tile([128, B], F32, tag="sel1")
            nc.vector.tensor_tensor(sel1[:101, :], acc1[0:101, :], ohb[:101, :], op=mybir.AluOpType.mult)

            partS_sb = spool.tile([1, 2 * B], F32, tag="pS")
            for h in range(2):
                s2 = mps.tile([128, 512], F32, tag="P")
                nc.tensor.matmul(
                    s2[0:1, :], ones_sb[:101, :], sel2[:101, h * 512:(h + 1) * 512],
                    start=True, stop=True,
                )
                nc.vector.tensor_copy(
                    partS_sb[0:1, h * 512:(h + 1) * 512], s2[0:1, :],
                )  # S2 half
                s1 = mps.tile([128, 512], F32, tag="P")
                nc.tensor.matmul(
                    s1[0:1, :], ones_sb[:101, :], sel1[:101, h * 512:(h + 1) * 512],
                    start=True, stop=True,
                )
                nc.vector.tensor_copy(
                    partS_sb[0:1, B + h * 512:B + (h + 1) * 512], s1[0:1, :],
                )
            nc.sync.dma_start(partS[:], partS_sb[:])

    nc.compile()
    return nc


def _prep_inputs(features, sup_logits, centers, labels):
    f = features.astype(np.float32)
    lab = labels.astype(np.int64)
    labB = lab[:B]
    ccount = np.bincount(lab, minlength=C).astype(np.float64)
    cntB = np.bincount(labB, minlength=C).astype(np.float64)
    cc1 = cntB + 1.0

    fbT = np.ascontiguousarray((f[:B] / T).T).astype(NP_BF16)          # [D, B]
    fT = f.T                                                           # [D, N]

    r0 = (1.0 / ccount[lab]).astype(np.float64)
    dv = 1.0 / (ccount[lab] - ALPHA) - r0
    lab1 = np.concatenate([labB, np.arange(C)])
    r0_1 = 1.0 / cc1[lab1]
    den1 = cc1[lab1] - 1.0
    d1 = np.where(den1 > 0, 1.0 / np.maximum(den1, 1e-30) - r0_1, 0.0)

    g2 = np.zeros((C, D), np.float64)
    np.add.at(g2, lab, f.astype(np.float64))
    g1 = np.zeros((C, D), np.float64)
    np.add.at(g1, labB, f[:B].astype(np.float64))
    g1 += centers.astype(np.float64)
    gT = np.concatenate([g2.T, g1.T], axis=1).astype(NP_BF16)          # [D, 200]

    esupT = np.exp(sup_logits.astype(np.float64)).T                    # [C, B]

    eye = np.eye(128, dtype=np.float32)
    iota_b = np.broadcast_to(np.arange(C, dtype=np.float32), (128, C)).copy()
    rowlab = labB.reshape(8, 128).T.astype(np.float32).copy()          # [p, chunk]
    labB_b = np.broadcast_to(labB.astype(np.float32), (128, B)).copy()
    iota_col = (np.arange(128, dtype=np.float32) - 1.0).reshape(128, 1).copy()

    def w_slot(col_lab, r0v, dvv, npart):
        w = np.zeros((128, 101), np.float64)
        w[:npart, 0] = r0v
        w[np.arange(npart), 1 + col_lab] = dvv
        return w

    in_maps = []
    for c in range(M):
        cols = np.zeros((D, N_SLOT * 128), np.float32)
        Wc = np.zeros((128, N_SLOT * 101), np.float64)
        sl = slice(c * SHARD, (c + 1) * SHARD)
        cols[:, :SHARD] = fT[:, sl]
        for jc in range(JC_B2):
            j0 = c * SHARD + jc * 128
            Wc[:, jc * 101:(jc + 1) * 101] = w_slot(
                lab[j0:j0 + 128], r0[j0:j0 + 128], dv[j0:j0 + 128], 128)
        # slot33: b1 batch-feature chunk c
        cols[:, JC_B2 * 128:(JC_B2 + 1) * 128] = fT[:, c * 128:(c + 1) * 128]
        Wc[:, JC_B2 * 101:(JC_B2 + 1) * 101] = w_slot(
            labB[c * 128:(c + 1) * 128], r0_1[c * 128:(c + 1) * 128],
            d1[c * 128:(c + 1) * 128], 128)
        mask = np.ones((128, 3 * B), np.float64)
        if c == 0:
            # slot34: centers chunk
            cols[:, (JC_B2 + 1) * 128:(JC_B2 + 1) * 128 + C] = centers.T
            Wc[:, (JC_B2 + 1) * 101:(JC_B2 + 2) * 101] = w_slot(
                np.arange(C), r0_1[B:], d1[B:], C)
            # slot35: sup logits, smuggled in via the mask (E = exp(0)*mask)
            Wc[:C, (JC_B2 + 2) * 101] = 1.0 / ccount
            Wc[np.arange(C), (JC_B2 + 2) * 101 + 1 + np.arange(C)] = (
                1.0 / (ccount - 1.0) - 1.0 / ccount)
            mask[:, 2 * B:] = 0.0
            mask[:C, 2 * B:] = esupT
            ad = 1.0 - eye
        else:
            ad = np.ones((128, 128), np.float32)
        # slot33 diag mask: b1 chunk c covers anchor rows [128c,128c+128)
        m33 = np.ones((128, B), np.float64)
        m33[:, c * 128:(c + 1) * 128] = 1.0 - eye
        mask[:, :B] = m33

        in_maps.append({
            "fbT": fbT,
            "fTs": cols.astype(NP_BF16),
            "W": Wc.astype(NP_BF16),
            "antidiag": ad.astype(NP_BF16),
            "maskfull": mask.astype(NP_BF16),
            "gT": gT,
            "iota_b": iota_b,
            "rowlab": rowlab,
            "labB": labB_b,
            "iota_col": iota_col,
        })
    return in_maps


def kernel(features, sup_logits, centers, labels, _debug=False, _trace=False):
    if "nc" not in _CACHE:
        _CACHE["nc"] = _build_nc()
    nc = _CACHE["nc"]
    in_maps = _prep_inputs(features, sup_logits, centers, labels)
    res = run_bass_kernel_spmd(nc, in_maps, core_ids=list(range(M)), trace=_trace)
    _CACHE["last"] = res

    lab = labels.astype(np.int64)
    labB = lab[:B]
    ccount = np.bincount(lab, minlength=C).astype(np.float64)
    cntB = np.bincount(labB, minlength=C).astype(np.float64)

    S2 = np.zeros(B, np.float64)
    S1 = np.zeros(B, np.float64)
    for c in range(M):
        S2 += res.results[c]["partS"][0, :B].astype(np.float64)
        S1 += res.results[c]["partS"][0, B:].astype(np.float64)
    pA = res.results[0]["partA"].astype(np.float64)
    A2 = pA[:, 0:8].T.reshape(B) - 1.0 / T
    A1 = pA[:, 8:16].T.reshape(B) - 1.0 / T

    N2 = ccount[labB] - 1.0
    msum = 1.0 + ALPHA * N2
    numer2 = sup_logits.astype(np.float64)[np.arange(B), labB] + ALPHA * A2
    loss2 = np.mean(np.log(S2) - numer2 / msum)
    N1 = cntB[labB]
    loss1 = np.mean(np.log(S1) - A1 / N1)
    return np.array(loss1 + loss2, dtype=np.float32)



# revision 2
# speedup vs baseline: 1.2919x; 1.2919x over previous
"""BPaCo+ loss on 8 TRN2 NeuronCores.

Flipped-layout design: anchors (i) live on PSUM partitions, contrast columns
(j) on the free axis. Each core owns a shard of the contrast columns:
  segA = batch chunk c (128 cols, branch 2)
  segQ = queue chunk c (4096 cols, branch 2)
  segC = batch chunk c again (128 cols, branch 1 weights)
  segD = centers chunk c (16 cols, 13 used, branch 1)

One fp8e4m3 DoubleRow matmul per PSUM tile computes, in a single pass,
  P[i,j] = f_i.f_j/T + ln(r0_j) + shift
by packing the 128 feature rows plus a rank-1 log-weight row (129 rows) into
65 partitions x 2 (DoubleRow virtualizes contraction to 130). The scalar
engine's exp activation then produces the softmax denominator partials via
accum_out (free-axis sum) -- no second reduction matmul, no vector work.
Diagonal masking adds -30000 into P[i,i] via a tiny identity matmul; branch-1
class weights ln(r01_j) + q1_j[lab1_j==lab_i] ride a small bf16 aug matmul.
Per-instance numerators A_i, the sup-logits block, and the final log/mean are
O(B*D) and run on host. Device returns [128, 32] accum partials per core.
"""
import numpy as np
import ml_dtypes

from concourse import bass, bacc, mybir, tile
from concourse.bass_utils import run_bass_kernel_spmd

B, K, C, D = 1024, 32768, 100, 128
N = B + K
T, ALPHA = 0.07, 0.05
M = 8                       # cores
QSH = K // M                # 4096 queue cols per core
NCTR = 16                   # centers slot per core (13 used)
NB2 = 128 + QSH             # 4224 branch-2 cols per core
NCOLS = NB2 + 128 + NCTR    # 4368 total rhs cols per core (x16 aligned)
IB = 8                      # anchor i-blocks of 128
DIAG_NEG = -30000.0

BF16 = mybir.dt.bfloat16
F32 = mybir.dt.float32
FP8 = mybir.dt.float8e4
NP_BF16 = ml_dtypes.bfloat16
NP_FP8 = ml_dtypes.float8_e4m3

_CACHE = {}


def _build_nc():
    nc = bacc.Bacc(None, target_bir_lowering=False)
    X = nc.declare_dram_parameter("X", [65, 2, IB * 128], FP8, isOutput=False)
    RQ = nc.declare_dram_parameter("RQ", [65, 2, NCOLS], FP8, isOutput=False)
    AUG1 = nc.declare_dram_parameter("AUG1", [101, IB * 128], BF16, isOutput=False)
    R1 = nc.declare_dram_parameter("R1", [101, 128 + NCTR], BF16, isOutput=False)
    EYET = nc.declare_dram_parameter("EYET", [128, 128], BF16, isOutput=False)
    DN = nc.declare_dram_parameter("DN", [128, IB * 128], BF16, isOutput=False)
    ACC = nc.declare_dram_parameter("ACC", [128, IB * 4], F32, isOutput=True)

    with tile.TileContext(nc) as tc:
        with (
            tc.tile_pool(name="const", bufs=1) as cpool,
            tc.tile_pool(name="scratch", bufs=2) as spool,
            tc.tile_pool(name="bigps", bufs=2, space=bass.MemorySpace.PSUM) as bps,
            tc.tile_pool(name="smallps", bufs=2, space=bass.MemorySpace.PSUM) as sps,
        ):
            X_sb = cpool.tile([65, 2, IB * 128], FP8, tag="X")
            nc.sync.dma_start(X_sb[:], X[:])
            RQ_sb = cpool.tile([65, 2, NCOLS], FP8, tag="RQ")
            nc.sync.dma_start(RQ_sb[:], RQ[:])
            EYET_sb = cpool.tile([128, 128], BF16, tag="EYET")
            nc.sync.dma_start(EYET_sb[:], EYET[:])
            DN_sb = cpool.tile([128, IB * 128], BF16, tag="DN")
            nc.sync.dma_start(DN_sb[:], DN[:])
            AUG1_sb = cpool.tile([101, IB * 128], BF16, tag="AUG1")
            nc.sync.dma_start(AUG1_sb[:], AUG1[:])
            R1_sb = cpool.tile([101, 128 + NCTR], BF16, tag="R1")
            nc.sync.dma_start(R1_sb[:], R1[:])

            ACC_sb = cpool.tile([128, IB * 4], F32, tag="ACCsb")

            DR = mybir.MatmulPerfMode.DoubleRow
            for b in range(IB):
                lhs = X_sb[:, :, b * 128:(b + 1) * 128]
                # --- branch-2 tiles: segA+queue = cols 0..4224 of RQ ---
                tiles = []
                for t0, t1 in ((0, 1536), (1536, 3072), (3072, NB2)):
                    P = bps.tile([128, 1536], F32, tag="P")
                    w = t1 - t0
                    c0 = 0
                    while c0 < w:
                        cw = min(512, w - c0)
                        nc.tensor.matmul(
                            P[:, c0:c0 + cw],
                            lhs,
                            RQ_sb[:, :, t0 + c0:t0 + c0 + cw],
                            start=True, stop=not (t0 == 0 and c0 == 0),
                            perf_mode=DR,
                        )
                        c0 += cw
                    tiles.append((P, w))
                # diagonal kill for segA (cols 0:128 of tile 0); DN slice is
                # -30000*eye only on the core's own i-block, zero elsewhere
                nc.tensor.matmul(
                    tiles[0][0][:, 0:128],
                    EYET_sb[:],
                    DN_sb[:, b * 128:(b + 1) * 128],
                    start=False, stop=True,
                )
                # --- branch-1 tile: segC+segD = cols 4224..4368 ---
                SM = sps.tile([128, 512], F32, tag="SM")
                nc.tensor.matmul(
                    SM[:, 0:128 + NCTR],
                    lhs,
                    RQ_sb[:, :, NB2:NCOLS],
                    start=True, stop=False,
                    perf_mode=DR,
                )
                nc.tensor.matmul(
                    SM[:, 0:128 + NCTR],
                    AUG1_sb[:, b * 128:(b + 1) * 128],
                    R1_sb[:],
                    start=False, stop=False,
                )
                nc.tensor.matmul(
                    SM[:, 0:128],
                    EYET_sb[:],
                    DN_sb[:, b * 128:(b + 1) * 128],
                    start=False, stop=True,
                )
                # --- exp + denominator partial sums (accum_out) ---
                for t, (P, w) in enumerate(tiles):
                    E = spool.tile([128, 1536], BF16, tag="E")
                    nc.scalar.activation(
                        E[:, 0:w], P[:, 0:w],
                        mybir.ActivationFunctionType.Exp,
                        accum_out=ACC_sb[:, 4 * b + t:4 * b + t + 1],
                    )
                ESM = spool.tile([128, 512], BF16, tag="ESM")
                nc.scalar.activation(
                    ESM[:, 0:128 + NCTR], SM[:, 0:128 + NCTR],
                    mybir.ActivationFunctionType.Exp,
                    accum_out=ACC_sb[:, 4 * b + 3:4 * b + 4],
                )

            nc.sync.dma_start(ACC[:], ACC_sb[:])

    nc.compile()
    return nc


def _pack130(rows130):
    """[130, n] -> [65, 2, n] fp8 DoubleRow packing (row r -> (r//2, r%2))."""
    n = rows130.shape[1]
    return np.ascontiguousarray(
        rows130.reshape(65, 2, n)).astype(NP_FP8)


def _prep_inputs(features, sup_logits, centers, labels):
    f = features.astype(np.float64)
    lab = labels.astype(np.int64)
    labB = lab[:B]
    ccount = np.bincount(lab, minlength=C).astype(np.float64)
    cntB = np.bincount(labB, minlength=C).astype(np.float64)
    cc1 = cntB + 1.0

    lnr0 = -np.log(ccount)
    s2 = -np.median(lnr0[lab])
    lnr0p = lnr0[lab] + s2                      # [N] b2 column log-weights
    lnr01 = -np.log(cc1)
    s1 = -np.median(lnr01[labB])
    q1 = np.where(cc1 > 1.0, np.log(cc1 / np.maximum(cc1 - 1.0, 1e-30)), 0.0)

    fq = f.astype(NP_FP8).astype(np.float32)    # quantize once, reuse
    fTq = (f[:B] / T).astype(NP_FP8).astype(np.float32)

    # lhsT: rows = [f_i/T (128); ones; zero] for all 1024 anchors
    lx = np.zeros((130, B), np.float32)
    lx[:D] = fTq.T
    lx[D] = 1.0
    X = _pack130(lx)

    eye = np.eye(128, dtype=np.float32)
    in_maps = []
    for c in range(M):
        jA = slice(c * 128, (c + 1) * 128)
        jQ = slice(B + c * QSH, B + (c + 1) * QSH)
        ci0 = c * 13
        ctr_idx = np.arange(ci0, min(ci0 + 13, C))
        nctr = len(ctr_idx)

        rq = np.zeros((130, NCOLS), np.float32)
        rq[:D, 0:128] = fq[jA].T
        rq[D, 0:128] = lnr0p[jA]
        rq[:D, 128:NB2] = fq[jQ].T
        rq[D, 128:NB2] = lnr0p[jQ]
        rq[:D, NB2:NB2 + 128] = fq[jA].T          # segC: same features, w1 via aug
        rq[:D, NB2 + 128:NB2 + 128 + nctr] = (
            centers.astype(NP_FP8).astype(np.float32)[ctr_idx].T)

        r1 = np.zeros((101, 128 + NCTR), np.float32)
        r1[0, 0:128] = lnr01[labB[jA]] + s1
        r1[1 + labB[jA], np.arange(128)] = q1[labB[jA]]
        r1[0, 128:128 + nctr] = lnr01[ctr_idx] + s1
        r1[1 + ctr_idx, 128 + np.arange(nctr)] = q1[ctr_idx]
        r1[0, 128 + nctr:] = DIAG_NEG             # kill padded center cols

        aug1 = np.zeros((101, B), np.float32)
        aug1[0] = 1.0
        aug1[1 + labB, np.arange(B)] = 1.0

        dn = np.zeros((128, IB * 128), np.float32)
        dn[:, c * 128:(c + 1) * 128] = DIAG_NEG * eye

        in_maps.append({
            "X": X,
            "RQ": rq.reshape(65, 2, NCOLS).astype(NP_FP8),
            "AUG1": aug1.astype(NP_BF16),
            "R1": r1.astype(NP_BF16),
            "EYET": eye.astype(NP_BF16),
            "DN": dn.astype(NP_BF16),
        })
    return in_maps, (lab, labB, ccount, cntB, cc1, s2, s1)


def kernel(features, sup_logits, centers, labels, _debug=False, _trace=False):
    if "nc" not in _CACHE:
        _CACHE["nc"] = _build_nc()
    nc = _CACHE["nc"]
    in_maps, (lab, labB, ccount, cntB, cc1, s2, s1) = _prep_inputs(
        features, sup_logits, centers, labels)
    res = run_bass_kernel_spmd(nc, in_maps, core_ids=list(range(M)), trace=_trace)
    _CACHE["last"] = res

    acc = np.zeros((128, IB * 4), np.float64)
    for c in range(M):
        acc += res.results[c]["ACC"].astype(np.float64)
    cols = acc.T.reshape(IB, 4, 128)              # [iblock, tile, lane]
    S2 = (cols[:, 0] + cols[:, 1] + cols[:, 2]).reshape(B) * np.exp(-s2)
    S1 = cols[:, 3].reshape(B) * np.exp(-s1)

    f = features.astype(np.float64)
    sup = sup_logits.astype(np.float64)
    oh = labB[:, None] == np.arange(C)[None, :]
    S2 = S2 + (np.exp(sup) / (ccount[None, :] - oh)).sum(1)

    g2 = np.zeros((C, D))
    np.add.at(g2, lab, f)
    g1 = np.zeros((C, D))
    np.add.at(g1, labB, f[:B])
    g1 += centers.astype(np.float64)
    A2 = np.einsum("id,id->i", f[:B], g2[labB]) / T - 1.0 / T
    A1 = np.einsum("id,id->i", f[:B], g1[labB]) / T - 1.0 / T

    msum = 1.0 + ALPHA * (ccount[labB] - 1.0)
    numer2 = sup[np.arange(B), labB] + ALPHA * A2
    loss2 = np.mean(np.log(S2) - numer2 / msum)
    loss1 = np.mean(np.log(S1) - A1 / cntB[labB])
    return np.array(loss1 + loss2, dtype=np.float32)


# revision 5
# speedup vs baseline: 1.5439x; 1.1951x over previous
"""BPaCo+ loss on 8 TRN2 NeuronCores.

Flipped-layout design: anchors (i) on PSUM partitions, queue columns (j) on
the free axis. Each core owns K/8 = 4096 queue columns. One fp8e4m3 DoubleRow
matmul per PSUM tile computes, in a single pass,
    P[i,j] = f_i.f_j/T + ln(r0_j) + shift
by packing the 128 feature rows plus a rank-1 log-weight row (129 rows) into
65 partitions x 2 (DoubleRow virtualizes contraction to 130). The scalar
engine exps [128, 2048] PSUM tiles into bf16 SBUF; the otherwise-idle vector
engine free-axis-reduces each exp tile into the softmax denominator partials.
No second matmul, no masks: queue columns are never diagonal and their
class-matched weight correction ln(c/(c-alpha)) ~ 1.5e-4 is dropped.

The O(B*(B+C)) blocks (batch-vs-batch, branch 1, sup logits) and the final
log/mean run on host in exact arithmetic. Device returns [128, 16] partial
sums per core; host gathers, adds its blocks, and assembles the loss.
"""
import numpy as np
import ml_dtypes

from concourse import bass, bacc, mybir, tile
from concourse.bass_utils import run_bass_kernel_spmd

B, K, C, D = 1024, 32768, 100, 128
T, ALPHA = 0.07, 0.05
M = 8                       # cores
QSH = K // M                # 4096 queue cols per core
IB = 8                      # anchor i-blocks of 128
HALF = QSH // 2             # 2048 cols per exp tile

BF16 = mybir.dt.bfloat16
F32 = mybir.dt.float32
FP8 = mybir.dt.float8e4
NP_BF16 = ml_dtypes.bfloat16
NP_FP8 = ml_dtypes.float8_e4m3

_CACHE = {}


def _build_nc():
    nc = bacc.Bacc(None, target_bir_lowering=False)
    X = nc.declare_dram_parameter("X", [65, 2, IB * 128], FP8, isOutput=False)
    RQ0 = nc.declare_dram_parameter("RQ0", [65, 2, HALF], FP8, isOutput=False)
    RQ1 = nc.declare_dram_parameter("RQ1", [65, 2, HALF], FP8, isOutput=False)
    ACC = nc.declare_dram_parameter("ACC", [128, 2 * IB], F32, isOutput=True)

    with tile.TileContext(nc) as tc:
        with (
            tc.tile_pool(name="const", bufs=1) as cpool,
            tc.tile_pool(name="scratch", bufs=3) as spool,
            tc.tile_pool(name="ps", bufs=2, space=bass.MemorySpace.PSUM) as pps,
        ):
            X_sb = cpool.tile([65, 2, IB * 128], FP8, tag="X")
            nc.sync.dma_start(X_sb[:], X[:])
            RQ0_sb = cpool.tile([65, 2, HALF], FP8, tag="RQ0")
            nc.sync.dma_start(RQ0_sb[:], RQ0[:])
            RQ1_sb = cpool.tile([65, 2, HALF], FP8, tag="RQ1")
            nc.sync.dma_start(RQ1_sb[:], RQ1[:])

            ACC_sb = cpool.tile([128, 2 * IB], F32, tag="ACCsb")

            DR = mybir.MatmulPerfMode.DoubleRow
            for ct, RQh in ((0, RQ0_sb), (1, RQ1_sb)):
                for b in range(IB):
                    lhs = X_sb[:, :, b * 128:(b + 1) * 128]
                    P = pps.tile([128, HALF], F32, tag="P")
                    for c0 in range(0, HALF, 512):
                        nc.tensor.matmul(
                            P[:, c0:c0 + 512],
                            lhs,
                            RQh[:, :, c0:c0 + 512],
                            start=True, stop=True,
                            perf_mode=DR,
                        )
                    E = spool.tile([128, HALF], BF16, tag="E")
                    nc.scalar.activation(
                        E[:], P[:], mybir.ActivationFunctionType.Exp,
                    )
                    col = 2 * b + ct
                    nc.vector.tensor_reduce(
                        ACC_sb[:, col:col + 1], E[:],
                        axis=mybir.AxisListType.X, op=mybir.AluOpType.add,
                    )

            nc.sync.dma_start(ACC[:], ACC_sb[:])

    nc.compile()
    return nc


def _prep_inputs(features, labels):
    f = features.astype(np.float64)
    lab = labels.astype(np.int64)
    ccount = np.bincount(lab, minlength=C).astype(np.float64)

    lnr0 = -np.log(ccount)
    s2 = -np.median(lnr0[lab])
    lnr0p = lnr0[lab] + s2

    fq = f.astype(NP_FP8).astype(np.float32)
    fTq = (f[:B] / T).astype(NP_FP8).astype(np.float32)

    lx = np.zeros((130, B), np.float32)
    lx[:D] = fTq.T
    lx[D] = 1.0
    X = np.ascontiguousarray(lx.reshape(65, 2, B)).astype(NP_FP8)

    in_maps = []
    for c in range(M):
        rq = np.zeros((130, QSH), np.float32)
        jQ = slice(B + c * QSH, B + (c + 1) * QSH)
        rq[:D] = fq[jQ].T
        rq[D] = lnr0p[jQ]
        rq = rq.reshape(65, 2, QSH).astype(NP_FP8)
        in_maps.append({
            "X": X,
            "RQ0": np.ascontiguousarray(rq[:, :, :HALF]),
            "RQ1": np.ascontiguousarray(rq[:, :, HALF:]),
        })
    return in_maps, s2


def kernel(features, sup_logits, centers, labels, _debug=False, _trace=False):
    if "nc" not in _CACHE:
        _CACHE["nc"] = _build_nc()
    nc = _CACHE["nc"]
    in_maps, s2 = _prep_inputs(features, labels)
    res = run_bass_kernel_spmd(nc, in_maps, core_ids=list(range(M)), trace=_trace)
    _CACHE["last"] = res

    acc = np.zeros((128, 2 * IB), np.float64)
    for c in range(M):
        acc += res.results[c]["ACC"].astype(np.float64)
    # lane p, col 2b+ct -> anchor i = 128b + p
    S2q = acc.reshape(128, IB, 2).sum(2).T.reshape(B) * np.exp(-s2)

    # ---- host blocks (exact): batch-vs-batch, branch 1, sup logits ----
    f = features.astype(np.float64)
    f32b = features.astype(np.float32)
    sup = sup_logits.astype(np.float64)
    lab = labels.astype(np.int64)
    labB = lab[:B]
    ccount = np.bincount(lab, minlength=C).astype(np.float64)
    cntB = np.bincount(labB, minlength=C).astype(np.float64)
    cc1 = cntB + 1.0

    cols = np.concatenate([f32b[:B], centers.astype(np.float32)], axis=0)  # [B+C, D]
    LG = (f32b[:B] @ cols.T) / np.float32(T)          # [B, B+C]
    ELG = np.exp(LG.astype(np.float64))
    ELG[np.arange(B), np.arange(B)] = 0.0             # diag masked in both branches

    match_bb = labB[:, None] == labB[None, :]
    # branch 2 batch block: w = ccount[lab_j] - ALPHA*match (off-diag)
    W2 = 1.0 / (ccount[labB][None, :] - ALPHA * match_bb)
    S2h = (ELG[:, :B] * W2).sum(1)
    # sup block
    oh = labB[:, None] == np.arange(C)[None, :]
    S2sup = (np.exp(sup) / (ccount[None, :] - oh)).sum(1)
    S2 = S2q + S2h + S2sup

    # branch 1: cols = [batch, centers], lab1 = [labB, 0..C-1]
    lab1 = np.concatenate([labB, np.arange(C)])
    match1 = labB[:, None] == lab1[None, :]
    W1 = 1.0 / (cc1[lab1][None, :] - match1)  # diag already zeroed in ELG
    S1 = (ELG * W1).sum(1)

    g2 = np.zeros((C, D))
    np.add.at(g2, lab, f)
    g1 = np.zeros((C, D))
    np.add.at(g1, labB, f[:B])
    g1 += centers.astype(np.float64)
    A2 = np.einsum("id,id->i", f[:B], g2[labB]) / T - 1.0 / T
    A1 = np.einsum("id,id->i", f[:B], g1[labB]) / T - 1.0 / T

    msum = 1.0 + ALPHA * (ccount[labB] - 1.0)
    numer2 = sup[np.arange(B), labB] + ALPHA * A2
    loss2 = np.mean(np.log(S2) - numer2 / msum)
    loss1 = np.mean(np.log(S1) - A1 / cntB[labB])
    return np.array(loss1 + loss2, dtype=np.float32)


# revision 12
# speedup vs baseline: 1.6738x; 1.0841x over previous
"""BPaCo+ loss on 8 TRN2 NeuronCores.

Flipped-layout design: anchors (i) on PSUM partitions, queue columns (j) on
the free axis. Each core owns K/8 = 4096 queue columns. One fp8e4m3 DoubleRow
matmul per PSUM bank computes, in a single pass,
    P[i,j] = f_i.f_j/T + ln(r0_j) + shift
by packing the 128 feature rows plus a rank-1 log-weight row (129 rows) into
65 partitions x 2 (DoubleRow virtualizes the contraction to 130). The scalar
engine exps [128, 2048] PSUM tiles into bf16 SBUF; the vector and gpsimd
engines each free-axis-reduce half of every exp tile into the softmax
denominator partials (the final tile instead uses the activation accumulator
so the tail is one DMA away). Queue columns are never diagonal and their
class-matched weight correction ln(c/(c-alpha)) ~ 1.5e-4 is dropped.

The O(B*(B+C)) blocks (batch-vs-batch, branch 1, sup logits) and the final
log/mean run on host in exact arithmetic. Device returns [128, 32] partial
sums per core; host gathers, adds its blocks, and assembles the loss.
"""
import numpy as np
import ml_dtypes

from concourse import bass, bacc, mybir, tile
from concourse.bass_utils import run_bass_kernel_spmd

B, K, C, D = 1024, 32768, 100, 128
T, ALPHA = 0.07, 0.05
M = 8                       # cores
QSH = K // M                # 4096 queue cols per core
IB = 8                      # anchor i-blocks of 128
TLE = 2048                  # exp tile columns
NT = QSH // TLE             # 2 col-tiles per iblock
CH = 1024                   # DMA chunk columns

BF16 = mybir.dt.bfloat16
F32 = mybir.dt.float32
FP8 = mybir.dt.float8e4
NP_BF16 = ml_dtypes.bfloat16
NP_FP8 = ml_dtypes.float8_e4m3

_CACHE = {}


def _build_nc():
    nc = bacc.Bacc(None, target_bir_lowering=False)
    X = nc.declare_dram_parameter("X", [65, 2, IB * 128], FP8, isOutput=False)
    RQ = [
        nc.declare_dram_parameter(f"RQ{k}", [65, 2, CH], FP8, isOutput=False)
        for k in range(QSH // CH)
    ]
    ACC = nc.declare_dram_parameter("ACC", [128, 16], F32, isOutput=True)

    with tile.TileContext(nc) as tc:
        with (
            tc.tile_pool(name="sb", bufs=1) as sbp,
            tc.tile_pool(name="ps", bufs=2, space=bass.MemorySpace.PSUM) as pps,
        ):
            # parallel DMA issue: X first (feeds ldweights), then chunks,
            # spread across engine queues so issue overhead doesn't serialize
            X_sb = sbp.tile([65, 2, IB * 128], FP8, tag="X")
            nc.sync.dma_start(X_sb[:], X[:])
            RQ_sb = []
            issuers = [nc.gpsimd, nc.scalar, nc.sync, nc.gpsimd]
            for k in range(QSH // CH):
                t = sbp.tile([65, 2, CH], FP8, tag=f"RQ{k}", name=f"RQ{k}_sb")
                issuers[k].dma_start(t[:], RQ[k][:])
                RQ_sb.append(t)

            ACC_sb = sbp.tile([128, 16], F32, tag="ACCsb")
            warm = sbp.tile([128, 1], F32, tag="warm")
            nc.gpsimd.memset(warm[:], 0.0)
            # pre-load the Exp activation table while DMAs are in flight
            nc.scalar.activation(
                warm[:], warm[:], mybir.ActivationFunctionType.Exp)

            DR = mybir.MatmulPerfMode.DoubleRow
            for t in range(NT * IB):
                ct, b = t // IB, t % IB
                lhs = X_sb[:, :, b * 128:(b + 1) * 128]
                P = pps.tile([128, TLE], F32, tag="P")
                for h in range(TLE // CH):
                    RQh = RQ_sb[ct * (TLE // CH) + h]
                    for c0 in range(0, CH, 512):
                        nc.tensor.matmul(
                            P[:, h * CH + c0:h * CH + c0 + 512],
                            lhs,
                            RQh[:, :, c0:c0 + 512],
                            start=True, stop=True,
                            perf_mode=DR,
                        )
                E = sbp.tile([128, TLE], BF16, tag="E", bufs=3)
                use_accum = t % 2 == 1
                nc.scalar.activation(
                    E[:], P[:], mybir.ActivationFunctionType.Exp,
                    accum_out=ACC_sb[:, t:t + 1] if use_accum else None,
                )
                if not use_accum:
                    nc.vector.tensor_reduce(
                        ACC_sb[:, t:t + 1], E[:],
                        axis=mybir.AxisListType.X, op=mybir.AluOpType.add,
                    )

            nc.sync.dma_start(ACC[:], ACC_sb[:])

    nc.compile()
    return nc


def _prep_inputs(features, labels):
    f = features.astype(np.float64)
    lab = labels.astype(np.int64)
    ccount = np.bincount(lab, minlength=C).astype(np.float64)

    lnr0 = -np.log(ccount)
    s2 = -np.median(lnr0[lab])
    lnr0p = lnr0[lab] + s2

    fq = f.astype(NP_FP8).astype(np.float32)
    fTq = (f[:B] / T).astype(NP_FP8).astype(np.float32)

    lx = np.zeros((130, B), np.float32)
    lx[:D] = fTq.T
    lx[D] = 1.0
    X = np.ascontiguousarray(lx.reshape(65, 2, B)).astype(NP_FP8)

    in_maps = []
    for c in range(M):
        rq = np.zeros((130, QSH), np.float32)
        jQ = slice(B + c * QSH, B + (c + 1) * QSH)
        rq[:D] = fq[jQ].T
        rq[D] = lnr0p[jQ]
        rq = rq.reshape(65, 2, QSH).astype(NP_FP8)
        im = {"X": X}
        for k in range(QSH // CH):
            im[f"RQ{k}"] = np.ascontiguousarray(rq[:, :, k * CH:(k + 1) * CH])
        in_maps.append(im)
    return in_maps, s2


def kernel(features, sup_logits, centers, labels, _debug=False, _trace=False):
    if "nc" not in _CACHE:
        _CACHE["nc"] = _build_nc()
    nc = _CACHE["nc"]
    in_maps, s2 = _prep_inputs(features, labels)
    res = run_bass_kernel_spmd(nc, in_maps, core_ids=list(range(M)), trace=_trace)
    _CACHE["last"] = res

    acc = np.zeros((128, 16), np.float64)
    for c in range(M):
        acc += res.results[c]["ACC"].astype(np.float64)
    # tile t = ct*IB + b covers anchors i = 128b+p
    per_block = acc[:, 0:IB] + acc[:, IB:2 * IB]  # [128 lane, 8 iblock]
    S2q = per_block.T.reshape(B) * np.exp(-s2)

    # ---- host blocks (exact): batch-vs-batch, branch 1, sup logits ----
    f = features.astype(np.float64)
    f32b = features.astype(np.float32)
    sup = sup_logits.astype(np.float64)
    lab = labels.astype(np.int64)
    labB = lab[:B]
    ccount = np.bincount(lab, minlength=C).astype(np.float64)
    cntB = np.bincount(labB, minlength=C).astype(np.float64)
    cc1 = cntB + 1.0

    cols = np.concatenate([f32b[:B], centers.astype(np.float32)], axis=0)  # [B+C, D]
    LG = (f32b[:B] @ cols.T) / np.float32(T)          # [B, B+C]
    ELG = np.exp(LG.astype(np.float64))
    ELG[np.arange(B), np.arange(B)] = 0.0             # diag masked in both branches

    match_bb = labB[:, None] == labB[None, :]
    W2 = 1.0 / (ccount[labB][None, :] - ALPHA * match_bb)
    S2h = (ELG[:, :B] * W2).sum(1)
    oh = labB[:, None] == np.arange(C)[None, :]
    S2sup = (np.exp(sup) / (ccount[None, :] - oh)).sum(1)
    S2 = S2q + S2h + S2sup

    lab1 = np.concatenate([labB, np.arange(C)])
    match1 = labB[:, None] == lab1[None, :]
    W1 = 1.0 / (cc1[lab1][None, :] - match1)  # diag already zeroed in ELG
    S1 = (ELG * W1).sum(1)

    g2 = np.zeros((C, D))
    np.add.at(g2, lab, f)
    g1 = np.zeros((C, D))
    np.add.at(g1, labB, f[:B])
    g1 += centers.astype(np.float64)
    A2 = np.einsum("id,id->i", f[:B], g2[labB]) / T - 1.0 / T
    A1 = np.einsum("id,id->i", f[:B], g1[labB]) / T - 1.0 / T

    msum = 1.0 + ALPHA * (ccount[labB] - 1.0)
    numer2 = sup[np.arange(B), labB] + ALPHA * A2
    loss2 = np.mean(np.log(S2) - numer2 / msum)
    loss1 = np.mean(np.log(S1) - A1 / cntB[labB])
    return np.array(loss1 + loss2, dtype=np.float32)
